# revision 19
# baseline (speedup 1.0000x reference)
"""Mean neighbor-aggregator (segment_reduce) for TRN2, 8 NeuronCores.

out[n, :] = mean_k weight[neighbor_idx[n, k], :]      n in [0, 100000), K=10

Data-parallel over nodes: each core owns 12500 nodes (padded to 12544 =
128*98 = 98 node-tiles) plus a replicated bf16 copy of the table.

Device algorithm per core (dma_gather is limited to int16 indices, so the
100000-row table is split into 4 chunks of 25000 rows):
  - dma_gather writes position i to SBUF partition i%128, slot i//128, so
    each 128-position group is a [128pos, 128d] bf16 tile; one dma_gather
    per (superbatch of 7 tiles, chunk), one SWDGE queue per chunk.
  - DVE builds one-hot selection matrices A[pos, node] = (nid[pos]==node)
    by comparing a host-supplied per-position node-id lane against an
    iota constant.
  - PE accumulates out_tile[node, d] = sum_g A_g.T @ G_g in PSUM (f32),
    ACT scales by 1/K into a bf16 output batch, DMA to DRAM ([P, NT, D]
    layout so the epilogue writes are large and contiguous).

The default path (VB) eliminates gather padding: instead of padding every
(tile, chunk) bucket to the worst case (budget 384 -> 20% extra
descriptors), each tile gets a static per-chunk group-count class
(A=(3,3,2,2), B=(2,2,3,3), C=(3,3,3,3); 43/43/12 tiles) and the host
vector-bin-packs nodes into tiles so every bucket fits its class caps
(2.8% padding, 128512 descriptors/core). Nodes that fit nowhere are split
across two tiles and their two partial rows summed on the host. If
packing fails, kernel() falls back to the uniform-budget path.

Measured bottleneck (neuron-profile): the four SWDGE gather queue-pipes
are the serial resource (~2ns/descriptor aggregate: ~8ns/desc per queue
for desc-gen + 16 shared DMA engines at ~14ns/256B packet, further duty-
throttled by the HW activity monitor). Desc count is therefore the main
lever; DVE/PE/ACT and the input/output DMAs all hide under the gather.
Things measured NOT to help: single_packet=True (device error),
prepare_only+trigger_dma (trigger inherits the block), GSPLIT, queue
rotation, bigger SWDGE scratch, 2 queues (2x worse), bigger superbatches.
"""

import os
import numpy as np
import ml_dtypes

import concourse.bacc as bacc
import concourse.bass as bass
import concourse.mybir as mybir
import concourse.tile as tile
from concourse.bass_utils import run_bass_kernel_spmd

N_NODES = 100000
K = 10
VOCAB = 100000
D = 128
NCORES = 8
PER_CORE = N_NODES // NCORES  # 12500
P = 128
NT = 98  # node-tiles per core (12544 nodes padded)
NCHK = 4
CHK = VOCAB // NCHK  # 25000
SBT = 7  # node-tiles per superbatch
NSB = NT // SBT  # 14

# --- experiment knobs (env-overridable; defaults = tuned config) ---
SP = bool(int(os.environ.get("K_SP", "0")))          # single_packet
SPLITLOAD = bool(int(os.environ.get("K_SPLITLOAD", "0")))  # per-sb idx loads
OBATCH = bool(int(os.environ.get("K_OBATCH", "0")))  # batched bf16 output
GSPLIT = int(os.environ.get("K_GSPLIT", "1"))        # gathers per (sb, q)
SCRATCH = int(os.environ.get("K_SCRATCH", "65536"))
GBUFS = int(os.environ.get("K_GBUFS", "3"))
ABUFS = int(os.environ.get("K_ABUFS", "3"))
QROT = bool(int(os.environ.get("K_QROT", "0")))      # rotate chunk->queue per sb
VB = bool(int(os.environ.get("K_VB", "1")))          # variable per-tile budgets

# --- variable-budget static scheme: per-tile class patterns (groups/chunk) ---
_CLS_A = (3, 3, 2, 2)
_CLS_B = (2, 2, 3, 3)
_CLS_C = (3, 3, 3, 3)


def _vb_scheme():
    """98 per-tile group-count tuples (43 A, 43 B, 12 C); per-sb patterns
    keep the four gather queues balanced."""
    pats = []
    for sb in range(NSB):
        if sb < 12:
            pats.append([_CLS_A, _CLS_B, _CLS_A, _CLS_B, _CLS_A, _CLS_B, _CLS_C])
        elif sb == 12:
            pats.append([_CLS_A, _CLS_B, _CLS_A, _CLS_B, _CLS_A, _CLS_B, _CLS_A])
        else:
            pats.append([_CLS_B, _CLS_A, _CLS_B, _CLS_A, _CLS_B, _CLS_A, _CLS_B])
    return [c for p in pats for c in p]


def _vb_layout():
    """Static layout derived from the scheme.

    Returns (G, RG, roff, toff, GB, GT_sb, TG):
      G[t][q]      groups of tile t, chunk q
      RG[sb][q]    groups in sb's chunk-q region
      roff[sb][q]  group offset of region q within sb's group space
      toff[sb][q][ti] tile ti's group offset within region q
      GB[sb]       global group base of sb
      GT_sb[sb]    groups in sb
      TG           total groups
    """
    G = _vb_scheme()
    RG, roff, toff, GB, GT_sb = [], [], [], [], []
    base = 0
    for sb in range(NSB):
        tiles = range(sb * SBT, (sb + 1) * SBT)
        rg = [sum(G[t][q] for t in tiles) for q in range(NCHK)]
        ro = np.concatenate([[0], np.cumsum(rg)]).astype(int)
        to = [
            np.concatenate([[0], np.cumsum([G[t][q] for t in tiles])]).astype(int)
            for q in range(NCHK)
        ]
        RG.append(rg)
        roff.append(ro)
        toff.append(to)
        GB.append(base)
        GT_sb.append(int(ro[-1]))
        base += int(ro[-1])
    return G, RG, roff, toff, GB, GT_sb, base

BF16 = ml_dtypes.bfloat16

_CACHE = {}


def _split_multi_waits(nc):
    """walrus codegen accepts a single sync wait per instruction; hoist
    extra waits onto standalone EventSemaphore insts on the same engine."""
    for f in nc.m.functions:
        for bb in f.blocks:
            new = []
            for inst in bb.instructions:
                si = inst.sync_info
                if si is not None and si.on_wait and len(si.on_wait) > 1:
                    waits = list(si.on_wait)
                    for w in waits[:-1]:
                        nop = mybir.InstEventSemaphore(
                            name=f"wsplit-{nc.next_id()}",
                            engine=inst.engine,
                            sync_info=mybir.SyncInfo(on_wait=[w], on_update=[]),
                            ins=[],
                            outs=[],
                        )
                        nc.register_instruction(nop)
                        new.append(nop)
                    inst.sync_info = mybir.SyncInfo(
                        on_wait=[waits[-1]], on_update=list(si.on_update or [])
                    )
                new.append(inst)
            bb.instructions = new


def build_vb():
    """Variable per-tile budget build: static ragged schedule from _vb_scheme."""
    f32, bf16, i16 = mybir.dt.float32, mybir.dt.bfloat16, mybir.dt.int16
    G, RG, roff, toff, GB, GT_sb, TG = _vb_layout()

    nc = bacc.Bacc("TRN2", num_swdge_queues=4, dynamic_dma_scratch_size=SCRATCH)
    w_ext = nc.declare_dram_parameter("weight", [VOCAB, D], bf16, isOutput=False)
    gidx_ext = nc.declare_dram_parameter("gidx", [P, TG * 8], i16, isOutput=False)
    nid_ext = nc.declare_dram_parameter("nid", [P, TG], bf16, isOutput=False)
    iota_ext = nc.declare_dram_parameter("iota", [P, P], bf16, isOutput=False)
    out_ext = nc.declare_dram_parameter("out", [P, NT, D], bf16, isOutput=True)

    with tile.TileContext(nc) as tc:
        with (
            tc.tile_pool(name="cst", bufs=1) as c_pool,
            tc.tile_pool(name="gb", bufs=GBUFS) as g_pool,
            tc.tile_pool(name="ab", bufs=ABUFS) as a_pool,
            tc.tile_pool(name="ob", bufs=4) as o_pool,
            tc.tile_pool(name="ps", bufs=8, space="PSUM") as p_pool,
        ):
            # sb0's metadata first (unblocks the first gather), then the rest
            # as two big loads to avoid HWDGE per-DMA issue overhead
            idx0 = c_pool.tile([P, GT_sb[0] * 8], i16, name="idx0")
            nc.sync.dma_start(out=idx0[:], in_=gidx_ext[:, : GT_sb[0] * 8])
            nid0 = c_pool.tile([P, GT_sb[0]], bf16, name="nid0")
            nc.sync.dma_start(out=nid0[:], in_=nid_ext[:, : GT_sb[0]])
            iota_t = c_pool.tile([P, P], bf16, name="iota")
            nc.sync.dma_start(out=iota_t[:], in_=iota_ext[:, :])
            idxr = c_pool.tile([P, (TG - GT_sb[0]) * 8], i16, name="idxr")
            nc.sync.dma_start(out=idxr[:], in_=gidx_ext[:, GT_sb[0] * 8 :])
            nidr = c_pool.tile([P, TG - GT_sb[0]], bf16, name="nidr")
            nc.sync.dma_start(out=nidr[:], in_=nid_ext[:, GT_sb[0] :])

            ni_regs = {}
            for sb in range(NSB):
                if sb == 0:
                    idx_t, nid_t, goff = idx0, nid0, 0
                else:
                    idx_t, nid_t, goff = idxr, nidr, GB[sb] - GT_sb[0]
                g_t = g_pool.tile([P, GT_sb[sb], D], bf16, tag="g", name=f"g{sb}")
                for q in range(NCHK):
                    rg = RG[sb][q]
                    o0 = int(roff[sb][q])
                    ni = rg * P
                    if ni not in ni_regs:
                        ni_regs[ni] = nc.gpsimd.to_reg(ni)
                    nc.gpsimd.dma_gather(
                        g_t[:, o0 : o0 + rg, :],
                        w_ext[q * CHK : (q + 1) * CHK, :],
                        idx_t[:, (goff + o0) * 8 : (goff + o0 + rg) * 8],
                        ni,
                        ni_regs[ni],
                        D,
                        single_packet=False,
                        queue_num=(q + sb) % NCHK if QROT else q,
                    )

                a_t = a_pool.tile([P, GT_sb[sb], P], bf16, tag="a", name=f"a{sb}")
                for q in range(NCHK):
                    rg = RG[sb][q]
                    o0 = int(roff[sb][q])
                    nc.vector.tensor_tensor(
                        out=a_t[:, o0 : o0 + rg, :],
                        in0=nid_t[:, goff + o0 : goff + o0 + rg]
                        .unsqueeze(2)
                        .to_broadcast([P, rg, P]),
                        in1=iota_t[:].unsqueeze(1).to_broadcast([P, rg, P]),
                        op=mybir.AluOpType.is_equal,
                    )

                ob_t = o_pool.tile([P, SBT, D], bf16, tag="o", name=f"ob{sb}")
                for ti in range(SBT):
                    t = sb * SBT + ti
                    gtt = sum(G[t])
                    ps_t = p_pool.tile([P, D], f32, tag="ps", name=f"ps{t}")
                    n_mm = 0
                    for q in range(NCHK):
                        for g in range(G[t][q]):
                            gi = int(roff[sb][q] + toff[sb][q][ti]) + g
                            nc.tensor.matmul(
                                ps_t[:],
                                lhsT=a_t[:, gi, :],
                                rhs=g_t[:, gi, :],
                                start=(n_mm == 0),
                                stop=(n_mm == gtt - 1),
                            )
                            n_mm += 1
                    nc.scalar.activation(
                        out=ob_t[:, ti, :],
                        in_=ps_t[:],
                        func=mybir.ActivationFunctionType.Copy,
                        scale=1.0 / K,
                    )
                nc.sync.dma_start(
                    out=out_ext[:, sb * SBT : (sb + 1) * SBT, :], in_=ob_t[:]
                )

    nc.compile()
    _split_multi_waits(nc)
    return nc


def build(budgets):
    """budgets: tuple of 4 ints (multiple of 128), slots per (tile, chunk)."""
    f32, bf16, i16 = mybir.dt.float32, mybir.dt.bfloat16, mybir.dt.int16
    B = list(budgets)
    G = [b // P for b in B]  # groups per (tile, chunk)
    GT = sum(G)  # groups per tile
    PSB = SBT * P * GT  # positions per superbatch
    # group offset of chunk q's region within a superbatch's position space
    qgoff = np.concatenate([[0], np.cumsum([SBT * g for g in G])]).astype(int)

    nc = bacc.Bacc("TRN2", num_swdge_queues=4, dynamic_dma_scratch_size=SCRATCH)
    w_ext = nc.declare_dram_parameter("weight", [VOCAB, D], bf16, isOutput=False)
    gidx_ext = nc.declare_dram_parameter(
        "gidx", [NSB, P, PSB // 16], i16, isOutput=False
    )
    nid_ext = nc.declare_dram_parameter(
        "nid", [NSB, P, PSB // P], bf16, isOutput=False
    )
    iota_ext = nc.declare_dram_parameter("iota", [P, P], bf16, isOutput=False)
    if OBATCH:
        out_ext = nc.declare_dram_parameter("out", [P, NT, D], bf16, isOutput=True)
    else:
        out_ext = nc.declare_dram_parameter("out", [NT, P, D], f32, isOutput=True)

    with tile.TileContext(nc) as tc:
        with (
            tc.tile_pool(name="cst", bufs=1) as c_pool,
            tc.tile_pool(name="gb", bufs=GBUFS) as g_pool,
            tc.tile_pool(name="ab", bufs=ABUFS) as a_pool,
            tc.tile_pool(name="ob", bufs=(4 if OBATCH else 8)) as o_pool,
            tc.tile_pool(name="ps", bufs=4, space="PSUM") as p_pool,
        ):
            iota_t = c_pool.tile([P, P], bf16, name="iota")
            nc.sync.dma_start(out=iota_t[:], in_=iota_ext[:, :])
            if SPLITLOAD:
                idx_sb = []
                nid_sb = []
                for sb in range(NSB):
                    it = c_pool.tile([P, PSB // 16], i16, name=f"idx{sb}")
                    nc.sync.dma_start(out=it[:], in_=gidx_ext[sb])
                    idx_sb.append(it)
                    ntl = c_pool.tile([P, PSB // P], bf16, name=f"nid{sb}")
                    nc.sync.dma_start(out=ntl[:], in_=nid_ext[sb])
                    nid_sb.append(ntl)
            else:
                # all superbatches' indices and node-ids in one prologue load
                idx_all = c_pool.tile([P, NSB * (PSB // 16)], i16, name="idxall")
                nc.sync.dma_start(
                    out=idx_all[:],
                    in_=gidx_ext[:, :, :].transpose([1, 0, 2]),
                )
                nid_all = c_pool.tile([P, NSB * (PSB // P)], bf16, name="nidall")
                nc.sync.dma_start(
                    out=nid_all[:],
                    in_=nid_ext[:, :, :].transpose([1, 0, 2]),
                )

            for sb in range(NSB):
                if SPLITLOAD:
                    idx_src, ixo = idx_sb[sb], 0
                    nid_src, ndo = nid_sb[sb], 0
                else:
                    idx_src, ixo = idx_all, sb * (PSB // 16)
                    nid_src, ndo = nid_all, sb * (PSB // P)
                g_t = g_pool.tile([P, SBT * GT, D], bf16, tag="g", name=f"g{sb}")
                for q in range(NCHK):
                    ngq = qgoff[q + 1] - qgoff[q]  # groups in this chunk region
                    # split into GSPLIT sub-gathers along group boundaries
                    bounds = [
                        qgoff[q] + (ngq * s) // GSPLIT for s in range(GSPLIT + 1)
                    ]
                    for s in range(GSPLIT):
                        g0, g1 = bounds[s], bounds[s + 1]
                        if g1 <= g0:
                            continue
                        ni = (g1 - g0) * P
                        nc.gpsimd.dma_gather(
                            g_t[:, g0:g1, :],
                            w_ext[q * CHK : (q + 1) * CHK, :],
                            idx_src[:, ixo + g0 * 8 : ixo + g1 * 8],
                            ni,
                            ni,
                            D,
                            single_packet=SP,
                            queue_num=(q + sb) % NCHK if QROT else q,
                        )

                # A[pos, node] = (nid[pos] == node), one op per chunk region
                a_t = a_pool.tile([P, SBT * GT, P], bf16, tag="a", name=f"a{sb}")
                for q in range(NCHK):
                    ng = qgoff[q + 1] - qgoff[q]
                    nc.vector.tensor_tensor(
                        out=a_t[:, qgoff[q] : qgoff[q + 1], :],
                        in0=nid_src[:, ndo + qgoff[q] : ndo + qgoff[q + 1]]
                        .unsqueeze(2)
                        .to_broadcast([P, ng, P]),
                        in1=iota_t[:].unsqueeze(1).to_broadcast([P, ng, P]),
                        op=mybir.AluOpType.is_equal,
                    )

                if OBATCH:
                    ob_t = o_pool.tile([P, SBT, D], bf16, tag="o", name=f"ob{sb}")
                for ti in range(SBT):
                    t = sb * SBT + ti
                    ps_t = p_pool.tile([P, D], f32, tag="ps", name=f"ps{t}")
                    n_mm = 0
                    for q in range(NCHK):
                        for g in range(G[q]):
                            gi = qgoff[q] + ti * G[q] + g
                            nc.tensor.matmul(
                                ps_t[:],
                                lhsT=a_t[:, gi, :],
                                rhs=g_t[:, gi, :],
                                start=(n_mm == 0),
                                stop=(n_mm == GT - 1),
                            )
                            n_mm += 1
                    if OBATCH:
                        nc.scalar.activation(
                            out=ob_t[:, ti, :],
                            in_=ps_t[:],
                            func=mybir.ActivationFunctionType.Copy,
                            scale=1.0 / K,
                        )
                    else:
                        o_t = o_pool.tile([P, D], f32, tag="o", name=f"o{t}")
                        nc.scalar.activation(
                            out=o_t[:],
                            in_=ps_t[:],
                            func=mybir.ActivationFunctionType.Copy,
                            scale=1.0 / K,
                        )
                        nc.sync.dma_start(out=out_ext[t], in_=o_t[:])
                if OBATCH:
                    nc.sync.dma_start(
                        out=out_ext[:, sb * SBT : (sb + 1) * SBT, :], in_=ob_t[:]
                    )

    nc.compile()
    _split_multi_waits(nc)
    return nc


def _budgets(neighbor_idx):
    """max bucket size over (core, tile, chunk), per chunk, rounded to 128."""
    nbr = np.asarray(neighbor_idx).astype(np.int64)
    v = nbr.reshape(NCORES, PER_CORE * K)
    node = np.arange(PER_CORE).repeat(K)
    t = node // P
    q = v // CHK  # [NCORES, 125000]
    maxc = np.zeros(NCHK, dtype=np.int64)
    for c in range(NCORES):
        key = t * NCHK + q[c]
        counts = np.bincount(key, minlength=NT * NCHK).reshape(NT, NCHK)
        maxc = np.maximum(maxc, counts.max(axis=0))
    return tuple(int(-(-m // P) * P) for m in maxc)


def shard_inputs(weight, neighbor_idx, budgets):
    w_bf16 = np.ascontiguousarray(np.asarray(weight, dtype=np.float32).astype(BF16))
    nbr = np.asarray(neighbor_idx).astype(np.int64)
    B = list(budgets)
    G = [b // P for b in B]
    GT = sum(G)
    PSB = SBT * P * GT
    iota = np.ascontiguousarray(
        np.broadcast_to(np.arange(P, dtype=np.float32), (P, P)).astype(BF16)
    )

    node = np.arange(PER_CORE).repeat(K)
    t_of = node // P
    m_of = (node % P).astype(np.int32)

    in_maps = []
    for core in range(NCORES):
        v = nbr[core * PER_CORE : (core + 1) * PER_CORE].reshape(-1)
        q = (v // CHK).astype(np.int32)
        lv = (v - q * CHK).astype(np.int16)
        key = t_of * NCHK + q
        order = np.argsort(key, kind="stable")
        ks, lvs, ms = key[order], lv[order], m_of[order]
        counts = np.bincount(ks, minlength=NT * NCHK).reshape(NT, NCHK)
        seg_end = np.cumsum(counts.reshape(-1)).reshape(NT, NCHK)

        gidx = np.zeros((NSB, PSB), np.int16)
        gnid = np.full((NSB, PSB), 255.0, np.float32)
        for sb in range(NSB):
            for qq in range(NCHK):
                base = SBT * P * int(np.concatenate([[0], np.cumsum(G)])[qq])
                for ti in range(SBT):
                    t = sb * SBT + ti
                    e = seg_end[t, qq]
                    s = e - counts[t, qq]
                    n = e - s
                    pos = base + ti * B[qq]
                    gidx[sb, pos : pos + n] = lvs[s:e]
                    gnid[sb, pos : pos + n] = ms[s:e]
        # wrap idx: position i -> [i%16, i//16], replicated across 8 groups
        gidx_w = np.tile(
            gidx.reshape(NSB, PSB // 16, 16).transpose(0, 2, 1), (1, 8, 1)
        )
        # nid lanes: position i -> [i%128, i//128]
        nid_l = gnid.reshape(NSB, PSB // P, P).transpose(0, 2, 1).astype(BF16)
        in_maps.append(
            {
                "weight": w_bf16,
                "gidx": np.ascontiguousarray(gidx_w),
                "nid": np.ascontiguousarray(nid_l),
                "iota": iota,
            }
        )
    return in_maps


def _pack_core(cnt, caps, seed=0):
    """Vector bin-packing of nodes (rows of cnt, [N,4] chunk counts) into
    len(caps) bins of 128 node slots with per-chunk slot caps caps[b][q].

    Returns (assign [N] bin ids, splits list of (i, b1, c1, b2) where node i's
    refs are split: c1[q] refs to b1, rest to b2) or None if packing failed.
    """
    NTl = len(caps)
    Ri = caps.astype(np.int64).copy()
    S = np.full(NTl, P, np.int64)
    N = len(cnt)
    assign = np.full(N, -1, np.int64)
    skew = np.abs(cnt - (K / NCHK)).sum(axis=1)
    rng = np.random.default_rng(seed)
    order = np.argsort(-(skew + rng.random(N) * 0.5), kind="stable")
    fails = []
    for i in order:
        c = cnt[i]
        ok = (S > 0) & (Ri >= c).all(axis=1)
        if not ok.any():
            fails.append(i)
            continue
        Rn = Ri / np.maximum(S[:, None], 1)
        score = (Rn * c).sum(axis=1)
        b = int(np.argmax(np.where(ok, score, -1e18)))
        assign[i] = b
        Ri[b] -= c
        S[b] -= 1
    # repair rounds: place each fail directly or via a single eviction
    for _ in range(3):
        if not fails:
            break
        still = []
        for i in fails:
            c = cnt[i]
            placed = False
            for b in np.argsort(-(Ri.sum(axis=1))):
                if S[b] > 0 and (Ri[b] >= c).all():
                    assign[i] = b
                    Ri[b] -= c
                    S[b] -= 1
                    placed = True
                    break
                members = np.where(assign == b)[0]
                if not len(members):
                    continue
                mc = cnt[members]
                cand = members[((Ri[b] + mc - c) >= 0).all(axis=1)]
                for m in cand:
                    ok2 = (S > 0) & (Ri >= cnt[m]).all(axis=1)
                    ok2[b] = False
                    if ok2.any():
                        b2 = int(
                            np.argmax(
                                np.where(ok2, (Ri * cnt[m]).sum(axis=1), -1e18)
                            )
                        )
                        assign[m] = b2
                        Ri[b2] -= cnt[m]
                        S[b2] -= 1
                        Ri[b] += cnt[m]  # m leaves b; i takes m's slot
                        assign[i] = b
                        Ri[b] -= c
                        placed = True
                        break
                if placed:
                    break
            if not placed:
                still.append(i)
        fails = still
    # last resort: split the node's refs across two bins (extra output row)
    splits = []
    for i in fails:
        c = cnt[i].astype(np.int64)
        done = False
        cand = np.where(S > 0)[0]
        for b1 in cand[np.argsort(-Ri[cand].sum(axis=1))]:
            c1 = np.minimum(c, Ri[b1])
            c2 = c - c1
            ok2 = (S > 0) & (Ri >= c2).all(axis=1)
            ok2[b1] = False
            if S[b1] > 0 and ok2.any():
                b2 = int(np.argmax(np.where(ok2, (Ri * c2).sum(axis=1), -1e18)))
                assign[i] = b1
                Ri[b1] -= c1
                S[b1] -= 1
                Ri[b2] -= c2
                S[b2] -= 1
                splits.append((int(i), int(b1), c1.copy(), int(b2)))
                done = True
                break
        if not done:
            return None
    return assign, splits


def shard_inputs_vb(weight, neighbor_idx):
    """Host prep for the variable-budget scheme.

    Returns (in_maps, gathers) where gathers[core] = (lane_of_ref arrays for
    unshard): per core (prim_tile[N], prim_lane[N], sec list[(node, tile, lane)]).
    Returns None if packing fails for any core (caller falls back).
    """
    G, RG, roff, toff, GB, GT_sb, TG = _vb_layout()
    caps = np.array(G) * P  # [98, 4] slot caps
    w_bf16 = np.ascontiguousarray(np.asarray(weight, dtype=np.float32).astype(BF16))
    nbr = np.asarray(neighbor_idx).astype(np.int64)
    iota = np.ascontiguousarray(
        np.broadcast_to(np.arange(P, dtype=np.float32), (P, P)).astype(BF16)
    )
    # static per-(tile, chunk) position bases in the flat TG*128 position space
    posbase = np.zeros((NT, NCHK), np.int64)
    for t in range(NT):
        sb, ti = divmod(t, SBT)
        for q in range(NCHK):
            posbase[t, q] = (GB[sb] + roff[sb][q] + toff[sb][q][ti]) * P

    in_maps, unshard_maps = [], []
    for core in range(NCORES):
        v = nbr[core * PER_CORE : (core + 1) * PER_CORE]  # [12500, 10]
        q = (v // CHK).astype(np.int32)
        cnt = np.zeros((PER_CORE, NCHK), np.int32)
        for c in range(NCHK):
            cnt[:, c] = (q == c).sum(axis=1)
        packed = None
        for seed in range(5):
            packed = _pack_core(cnt, caps, seed)
            if packed is not None:
                break
        if packed is None:
            return None
        assign, splits = packed

        # per-ref placement: start with primary bin for all refs
        ref_bin = np.repeat(assign, K).reshape(PER_CORE, K).copy()
        # virtual placements: (bin -> list of member placement ids)
        # placement = (node, bin); lanes assigned per bin in placement order
        split_secondary = {}
        for (i, b1, c1, b2) in splits:
            # move refs beyond c1[q] per chunk to b2
            used = np.zeros(NCHK, np.int64)
            for k in range(K):
                qq = q[i, k]
                if used[qq] < c1[qq]:
                    ref_bin[i, k] = b1
                    used[qq] += 1
                else:
                    ref_bin[i, k] = b2
            split_secondary[i] = b2

        # lane assignment per bin: primaries first (in node order), then secondaries
        lane_of_node = np.full(PER_CORE, -1, np.int64)
        tile_of_node = assign.copy()
        nlanes = np.zeros(NT, np.int64)
        for i in range(PER_CORE):
            b = assign[i]
            lane_of_node[i] = nlanes[b]
            nlanes[b] += 1
        sec_places = []  # (node, tile, lane)
        sec_lane = {}
        for (i, b1, c1, b2) in splits:
            sec_lane[i] = nlanes[b2]
            sec_places.append((int(i), int(b2), int(nlanes[b2])))
            nlanes[b2] += 1
        assert (nlanes <= P).all()

        # per-ref (tile, lane, chunk, lv)
        rt = ref_bin.reshape(-1)
        rlane = np.repeat(lane_of_node, K).reshape(PER_CORE, K)
        for (i, b1, c1, b2) in splits:
            mask = ref_bin[i] == b2
            rlane[i, mask] = sec_lane[i]
        rlane = rlane.reshape(-1)
        rq = q.reshape(-1)
        rlv = (v.reshape(-1) - rq.astype(np.int64) * CHK).astype(np.int16)

        # sort refs by (tile, chunk) and scatter into flat position space
        key = rt * NCHK + rq
        order = np.argsort(key, kind="stable")
        ks, lvs, lns = key[order], rlv[order], rlane[order]
        counts = np.bincount(ks, minlength=NT * NCHK).reshape(NT, NCHK)
        assert (counts <= caps).all()
        # rank within bucket
        seg_start = np.concatenate([[0], np.cumsum(counts.reshape(-1))[:-1]])
        rank = np.arange(len(ks)) - seg_start[ks]
        pos = posbase.reshape(-1)[ks] + rank

        gpos = np.zeros(TG * P, np.int16)
        gnid = np.full(TG * P, 255.0, np.float32)
        gpos[pos] = lvs
        gnid[pos] = lns

        # wrap per sb: idx [16, GT_sb*8] replicated x8; nid [128, GT_sb]
        idx_cols, nid_cols = [], []
        for sb in range(NSB):
            s0, s1 = GB[sb] * P, (GB[sb] + GT_sb[sb]) * P
            seg = gpos[s0:s1]
            idx_cols.append(seg.reshape(-1, 16).T)  # [16, GT_sb*8]
            nid_cols.append(gnid[s0:s1].reshape(-1, P).T)  # [128, GT_sb]
        gidx_w = np.tile(np.hstack(idx_cols), (8, 1))
        nid_l = np.hstack(nid_cols).astype(BF16)
        in_maps.append(
            {
                "weight": w_bf16,
                "gidx": np.ascontiguousarray(gidx_w),
                "nid": np.ascontiguousarray(nid_l),
                "iota": iota,
            }
        )
        unshard_maps.append((tile_of_node, lane_of_node, sec_places))
    return in_maps, unshard_maps


def unshard_output_vb(results, unshard_maps):
    outs = []
    for core in range(NCORES):
        o = np.asarray(results[core]["out"]).astype(np.float32)  # [P, NT, D]
        tile_of_node, lane_of_node, sec_places = unshard_maps[core]
        full = o[lane_of_node, tile_of_node]  # [12500, D]
        for (i, t, l) in sec_places:
            full[i] += o[l, t]
        outs.append(full)
    return np.concatenate(outs, axis=0)


def unshard_output(results):
    outs = []
    for core in range(NCORES):
        o = np.asarray(results[core]["out"])
        if OBATCH:
            # [P, NT, D] bf16 -> [NT*P, D] f32
            o = o.transpose(1, 0, 2).reshape(NT * P, D).astype(np.float32)
        else:
            o = o.reshape(NT * P, D)
        outs.append(o[:PER_CORE])
    return np.concatenate(outs, axis=0)


def _sample_check(out, weight, nbr):
    """Detect (rare, intermittent) corrupted runs by checking a node sample
    against a host-computed reference; the device path is bf16 (~2e-3), so a
    5e-2 gate cleanly separates rounding from corruption."""
    rng = np.random.default_rng(12345)
    idx = rng.choice(N_NODES, size=256, replace=False)
    w = np.asarray(weight, dtype=np.float32)
    exp = w[np.asarray(nbr)[idx].astype(np.int64)].mean(axis=1)
    denom = max(np.abs(exp).max(), 1e-6)
    return np.abs(out[idx] - exp).max() / denom < 5e-2


def kernel(weight, neighbor_idx):
    if VB:
        prep = shard_inputs_vb(weight, neighbor_idx)
        if prep is not None:
            in_maps, unshard_maps = prep
            nc = _CACHE.get("vb")
            if nc is None:
                nc = _CACHE["vb"] = build_vb()
            out = None
            for _ in range(3):
                res = run_bass_kernel_spmd(
                    nc, in_maps, core_ids=list(range(NCORES))
                )
                out = unshard_output_vb(res.results, unshard_maps)
                if _sample_check(out, weight, neighbor_idx):
                    return out
            return out
        # packing failed: fall through to uniform budgets
    budgets = _budgets(neighbor_idx)
    nc = _CACHE.get(budgets)
    if nc is None:
        nc = _CACHE[budgets] = build(budgets)
    in_maps = shard_inputs(weight, neighbor_idx, budgets)
    out = None
    for _ in range(3):
        res = run_bass_kernel_spmd(nc, in_maps, core_ids=list(range(NCORES)))
        out = unshard_output(res.results)
        if _sample_check(out, weight, neighbor_idx):
            break
    return out
